# revision 11
# baseline (speedup 1.0000x reference)
"""Trainium2 Bass kernel for nn_ContextPromptGenerator.

Math restructure: pooled bins are masked segment sums over tokens, so the
0/1-mask matmul runs FIRST on [T, 4096] packed rows, then the 4096->1024
down-projection runs on [64, 4096] per core.  All operands are plain fp16
(fp32 PSUM): the error gate is 2e-2 and fp16 lands ~4e-4.

Phase X uses the mask as the STATIONARY operand and streams x 512 columns
per matmul (KT*8 matmuls) instead of x-stationary/mask-moving (KT*32): the
per-instruction LDWEIGHTS+issue overhead dominated phase X, and the trace
showed X tensor-paced at ~3.5us per 128-token tile vs 2.8us of DMA.  The
[64, H] bin-sum rows are then PE-transposed back to [128h, 64] tiles for
the down-projection.

Phase order A -> B -> X keeps every PSUM pool sequential (psA 1 bank ->
psB 2 -> psX 8 -> psT/psD/psE/psU) and lets B's Wc stream land during A's
tensor-bound window.

DMA: all weights/masks are host-reordered partition-major so partition
lines are contiguous multi-KB runs (8KB packets sustain ~430GB/s vs
~170GB/s for 2KB rows); Wd is resident (64KB/partition), Wu double-buffered
by halves.

Sharding: data-parallel, 2 samples per core, paired to minimize
roundup128(max seq pair) + roundup128(max ctx pair).
"""

import numpy as np
from contextlib import ExitStack

import concourse.bass as bass
import concourse.mybir as mybir
import concourse.tile as tile
from concourse import bacc
from concourse.masks import make_identity
from concourse.bass_utils import run_bass_kernel_spmd

F32 = mybir.dt.float32
F16 = mybir.dt.float16

B, S, C, H, D, V, P = 16, 2048, 512, 4096, 1024, 32000, 32
NC = 8          # cores
SPC = 2         # samples per core
M = SPC * P     # 64 output rows per core
HT = H // 128   # 32 h-tiles
DT = D // 128   # 8 d-tiles

_cache = {}


def _build(T, Tc):
    """Build the per-core SPMD Bass program.

    T  = packed hidden rows per core (multiple of 128)
    Tc = packed context-embedding rows per core (multiple of 128)
    """
    nc = bacc.Bacc(None, target_bir_lowering=False)

    KT = T // 128    # x k-tiles
    KC = Tc // 128   # emb k-tiles

    xh_d = nc.dram_tensor("xh", [T, H], F16, kind="ExternalInput")
    mx_d = nc.dram_tensor("mxr", [128, KT * M], F16, kind="ExternalInput")
    eh_d = nc.dram_tensor("eh", [Tc, H], F16, kind="ExternalInput")
    cm_d = nc.dram_tensor("cmr", [128, KC * SPC], F16, kind="ExternalInput")
    wch_d = nc.dram_tensor("wcr", [128, HT * D], F16, kind="ExternalInput")
    wdh_d = nc.dram_tensor("wdr", [128, HT * D], F16, kind="ExternalInput")
    wuh_d = nc.dram_tensor("wur", [128, 2 * DT * H // 2], F16,
                           kind="ExternalInput")
    bd_d = nc.dram_tensor("bdr", [1, D], F32, kind="ExternalInput")
    bc_d = nc.dram_tensor("bcr", [1, D], F32, kind="ExternalInput")
    bu_d = nc.dram_tensor("bur", [1, H], F16, kind="ExternalInput")
    aug_d = nc.dram_tensor("aug", [4, M], F32, kind="ExternalInput")
    sinv_d = nc.dram_tensor("sinv", [M, 1], F32, kind="ExternalInput")
    out_d = nc.dram_tensor("out", [M, H], F32, kind="ExternalOutput")

    WCC = 8 * D       # Wc/Wd chunk cols (8 k-tiles per chunk)
    NCHUNK = HT // 8  # 4 chunks

    with tile.TileContext(nc) as tc, ExitStack() as ctx:
        const = ctx.enter_context(tc.tile_pool(name="const", bufs=1))
        big = ctx.enter_context(tc.tile_pool(name="big", bufs=2))
        wcpool = ctx.enter_context(tc.tile_pool(name="wcpool", bufs=1))
        wupool = ctx.enter_context(tc.tile_pool(name="wupool", bufs=2))
        opool = ctx.enter_context(tc.tile_pool(name="opool", bufs=1))
        keep = ctx.enter_context(tc.tile_pool(name="keep", bufs=1))

        ident = const.tile([128, 128], F32)
        make_identity(nc, ident)
        ident16 = const.tile([128, 128], F16)
        nc.vector.tensor_copy(ident16, ident)
        ones1 = const.tile([1, M], F16)
        nc.vector.memset(ones1, 1.0)
        aug_sb = keep.tile([4, M], F32)
        nc.sync.dma_start(out=aug_sb, in_=aug_d[:, :])
        sinv_sb = keep.tile([M, 1], F32)
        nc.sync.dma_start(out=sinv_sb, in_=sinv_d[:, :])
        # augmented-rhs rows: 0=ctxWcSum[a], 1=ctxWcSum[b], 2=bd, 3=bc
        augr_sb = keep.tile([4, D], F32)
        nc.sync.dma_start(out=augr_sb[2:3, :], in_=bd_d[:, :])
        nc.sync.dma_start(out=augr_sb[3:4, :], in_=bc_d[:, :])
        mxr_sb = keep.tile([128, KT * M], F16)
        nc.sync.dma_start(out=mxr_sb, in_=mx_d[:, :])
        cmr_sb = keep.tile([128, KC * SPC], F16)
        nc.sync.dma_start(out=cmr_sb, in_=cm_d[:, :])
        wd_sb = keep.tile([128, HT * D], F16)   # resident Wd, 64KB/part

        # ---- phase A: ctx_sumT[h, s] = sum_r emb[r, h] * cm01[r, s] ----
        cs_hi = keep.tile([128, HT * SPC], F16)
        with tc.tile_pool(name="psA", bufs=1, space="PSUM") as psA:
            ps_ctx = psA.tile([128, HT * SPC], F32)  # 1 bank
            for k in range(KC):
                eht = big.tile([128, H], F16, tag="hih")
                nc.sync.dma_start(out=eht, in_=eh_d[128 * k:128 * (k + 1), :])
                for hc in range(HT):
                    nc.tensor.matmul(
                        ps_ctx[:, SPC * hc:SPC * (hc + 1)],
                        eht[:, 128 * hc:128 * (hc + 1)],
                        cmr_sb[:, SPC * k:SPC * (k + 1)],
                        start=(k == 0 and hc == 0),
                        stop=(k == KC - 1),
                    )
            nc.vector.tensor_copy(cs_hi, ps_ctx)

        # ---- phase B: ctxWcSum rows [2, 1024] = cs.T @ Wc ----
        with tc.tile_pool(name="psB", bufs=1, space="PSUM") as psB:
            ps_cd = psB.tile([SPC, D], F32)       # 2 banks
            for c in range(NCHUNK):
                wct = wcpool.tile([128, WCC], F16, tag="wc")
                nc.sync.dma_start(out=wct, in_=wch_d[:, WCC * c:WCC * (c + 1)])
                for kk in range(8):
                    k = 8 * c + kk
                    for nb in range(2):
                        nc.tensor.matmul(
                            ps_cd[:, 512 * nb:512 * (nb + 1)],
                            cs_hi[:, SPC * k:SPC * (k + 1)],
                            wct[:, D * kk + 512 * nb:D * kk + 512 * (nb + 1)],
                            start=(k == 0),
                            stop=(k == HT - 1),
                        )
            nc.vector.tensor_copy(augr_sb[0:2, :], ps_cd)

        # ---- phase X: xsum[j, h] = sum_t mx01[t, j] * x[t, h] ----
        # mask stationary [128, 64], x moving 512-wide: KT*8 matmuls
        xsr = keep.tile([M, H], F16)   # bin sums, row layout
        with tc.tile_pool(name="psX", bufs=1, space="PSUM") as psX:
            ps_x = psX.tile([M, H], F32)  # 8 banks (partitions 0:64)
            for k in range(KT):
                xht = big.tile([128, H], F16, tag="hih")
                nc.sync.dma_start(out=xht, in_=xh_d[128 * k:128 * (k + 1), :])
                for nb in range(8):
                    nc.tensor.matmul(
                        ps_x[:, 512 * nb:512 * (nb + 1)],
                        mxr_sb[:, M * k:M * (k + 1)],
                        xht[:, 512 * nb:512 * (nb + 1)],
                        start=(k == 0),
                        stop=(k == KT - 1),
                    )
            for nb in range(8):
                nc.vector.tensor_copy(
                    xsr[:, 512 * nb:512 * (nb + 1)],
                    ps_x[:, 512 * nb:512 * (nb + 1)])

        # resident Wd lands while the transposes run
        for c in range(NCHUNK):
            nc.sync.dma_start(
                out=wd_sb[:, WCC * c:WCC * (c + 1)],
                in_=wdh_d[:, WCC * c:WCC * (c + 1)])

        # ---- transpose xsr -> xs_hi [128, 32*64] h-tile slices ----
        xs_hi = keep.tile([128, HT * M], F16)
        with tc.tile_pool(name="psT", bufs=2, space="PSUM") as psT:
            for k in range(HT):
                pst = psT.tile([128, M], F16, tag="xtr")
                nc.tensor.transpose(
                    pst, xsr[:, 128 * k:128 * (k + 1)],
                    ident16[0:M, 0:M])
                nc.vector.tensor_copy(xs_hi[:, M * k:M * (k + 1)], pst)

        # ---- phase D: pooled[j, d] = xsum.T @ Wd + aug ----
        silu_sb = keep.tile([M, D], F16)
        with tc.tile_pool(name="psD", bufs=1, space="PSUM") as psD:
            ps_pool = psD.tile([M, D], F32)  # 2 banks
            for k in range(HT):
                for nb in range(2):
                    nc.tensor.matmul(
                        ps_pool[:, 512 * nb:512 * (nb + 1)],
                        xs_hi[:, M * k:M * (k + 1)],
                        wd_sb[:, D * k + 512 * nb:D * k + 512 * (nb + 1)],
                        start=(k == 0),
                        stop=False,
                    )
            for nb in range(2):
                nc.tensor.matmul(
                    ps_pool[:, 512 * nb:512 * (nb + 1)],
                    aug_sb,
                    augr_sb[:, 512 * nb:512 * (nb + 1)],
                    start=False, stop=True,
                )
            # scale by 1/S and silu in one ACT op per bank
            for nb in range(2):
                nc.scalar.activation(
                    silu_sb[:, 512 * nb:512 * (nb + 1)],
                    ps_pool[:, 512 * nb:512 * (nb + 1)],
                    mybir.ActivationFunctionType.Silu,
                    scale=sinv_sb,
                )

        # ---- phase E: siluT slices [128, 64] per d-tile ----
        sT_hi = keep.tile([128, DT * M], F16)
        with tc.tile_pool(name="psE", bufs=2, space="PSUM") as psE:
            for dc in range(DT):
                pst = psE.tile([128, M], F16, tag="silutr")
                nc.tensor.transpose(
                    pst, silu_sb[:, 128 * dc:128 * (dc + 1)],
                    ident16[0:M, 0:M])
                nc.vector.tensor_copy(sT_hi[:, M * dc:M * (dc + 1)], pst)

        # ---- phase U: out[j, h] = siluT.T @ Wu + bu ----
        HH = H // 2  # two halves to keep psum at 4 banks
        with tc.tile_pool(name="psU", bufs=2, space="PSUM") as psU:
            for half in range(2):
                wut = wupool.tile([128, DT * HH], F16, tag="wuh")  # 32KB/part
                for c in range(2):
                    nc.sync.dma_start(
                        out=wut[:, DT * HH // 2 * c:DT * HH // 2 * (c + 1)],
                        in_=wuh_d[:, DT * HH * half + DT * HH // 2 * c:
                                  DT * HH * half + DT * HH // 2 * (c + 1)])
                but = opool.tile([1, HH], F16, tag="bu")
                nc.sync.dma_start(
                    out=but, in_=bu_d[:, HH * half:HH * (half + 1)])
                ps_out = psU.tile([M, HH], F32, tag="outps")  # 4 banks
                for dc in range(DT):
                    for nb in range(HH // 512):
                        nc.tensor.matmul(
                            ps_out[:, 512 * nb:512 * (nb + 1)],
                            sT_hi[:, M * dc:M * (dc + 1)],
                            wut[:, HH * dc + 512 * nb:HH * dc + 512 * (nb + 1)],
                            start=(dc == 0),
                            stop=False,
                        )
                for nb in range(HH // 512):
                    nc.tensor.matmul(
                        ps_out[:, 512 * nb:512 * (nb + 1)],
                        ones1,
                        but[:, 512 * nb:512 * (nb + 1)],
                        start=False, stop=True,
                    )
                ot = opool.tile([M, HH], F32, tag="ot")
                for nb in range(HH // 512):
                    nc.vector.tensor_copy(
                        ot[:, 512 * nb:512 * (nb + 1)],
                        ps_out[:, 512 * nb:512 * (nb + 1)])
                nc.sync.dma_start(
                    out=out_d[:, HH * half:HH * (half + 1)], in_=ot)

    nc.finalize()
    return nc


def _roundup(v, m):
    return max(m, ((int(v) + m - 1) // m) * m)


def _pm(a, kt):
    """Reorder [kt*128, cols] row-major -> partition-major [128, kt*cols]."""
    n, cols = a.shape
    assert n == kt * 128
    return np.ascontiguousarray(
        a.reshape(kt, 128, cols).transpose(1, 0, 2).reshape(128, kt * cols))


def _pair_samples(seq, clen):
    """Pair the 16 samples 2-per-core minimizing packed-row DMA:
    roundup128(max pair seq) + roundup128(max pair clen), tie-broken by joint
    row count.  Greedy sort-and-reflect on seq+clen, then 2-opt passes."""
    w = seq + clen
    order = np.argsort(-w, kind="stable")
    pairs = [[int(order[i]), int(order[2 * NC - 1 - i])] for i in range(NC)]

    def cost(ps):
        tx = max(seq[a] + seq[b] for a, b in ps)
        te = max(clen[a] + clen[b] for a, b in ps)
        return (_roundup(tx, 128) + _roundup(te, 128),
                max(seq[a] + clen[a] + seq[b] + clen[b] for a, b in ps))

    best = cost(pairs)
    improved = True
    while improved:
        improved = False
        for i in range(NC):
            for j in range(i + 1, NC):
                for swap in ((1, 1), (1, 0), (0, 1)):
                    cand = [list(p) for p in pairs]
                    cand[i][swap[0]], cand[j][swap[1]] = \
                        cand[j][swap[1]], cand[i][swap[0]]
                    c = cost(cand)
                    if c < best:
                        best, pairs, improved = c, cand, True
    return [(a, b) for a, b in pairs]


def kernel(**inputs):
    ids = np.asarray(inputs["context_ids"]).astype(np.int64)
    x = np.asarray(inputs["hidden_states"], dtype=np.float32)
    seq = np.asarray(inputs["seq_lengths"]).astype(np.int64)
    clen = np.asarray(inputs["context_lengths"]).astype(np.int64)
    emb = np.asarray(inputs["embed_table"], dtype=np.float32)
    Wc = np.ascontiguousarray(inputs["Wc"], dtype=np.float32)
    bc = np.asarray(inputs["bc"], dtype=np.float32)
    Wd = np.ascontiguousarray(inputs["Wd"], dtype=np.float32)
    bd = np.asarray(inputs["bd"], dtype=np.float32)
    Wu = np.ascontiguousarray(inputs["Wu"], dtype=np.float32)
    bu = np.asarray(inputs["bu"], dtype=np.float32)

    assert x.shape == (B, S, H) and ids.shape == (B, C)

    # per-sample bin geometry
    L = seq + 1
    jj = np.arange(P, dtype=np.int64)
    start = (jj[None, :] * L[:, None]) // P            # [B,P]
    end = ((jj[None, :] + 1) * L[:, None] + P - 1) // P
    Sj = (end - start).astype(np.float32)
    lo = np.maximum(start - 1, 0)
    hi = end - 1
    cnt = (hi - lo).astype(np.float32)
    ind = (start == 0).astype(np.float32)

    pairs = _pair_samples(seq, clen)
    T = _roundup(max(seq[a] + seq[b] for a, b in pairs), 128)
    Tc = _roundup(max(clen[a] + clen[b] for a, b in pairs), 128)
    KT, KC = T // 128, Tc // 128

    key = (T, Tc)
    if key not in _cache:
        _cache[key] = _build(T, Tc)
    nc = _cache[key]

    # partition-major weight layouts (8KB+ DMA packets)
    wcr = _pm(Wc.astype(np.float16), HT)
    wdr = _pm(Wd.astype(np.float16), HT)
    # Wu: [1024, 4096] -> [128, (half, dc, 2048)]
    wur = np.ascontiguousarray(
        Wu.astype(np.float16).reshape(DT, 128, 2, H // 2)
        .transpose(1, 2, 0, 3).reshape(128, 2 * DT * (H // 2)))
    bd_r = bd.reshape(1, D)
    bc_r = bc.reshape(1, D)
    bu_r = bu.astype(np.float16).reshape(1, H)
    emb16 = emb.astype(np.float16)

    in_maps = []
    for a, b in pairs:
        sa, sb = int(seq[a]), int(seq[b])
        ca, cb = max(1, int(clen[a])), max(1, int(clen[b]))
        xp = np.zeros((T, H), np.float16)
        xp[:sa] = x[a, :sa]
        xp[sa:sa + sb] = x[b, :sb]
        t = np.arange(T, dtype=np.int64)[:, None]
        mx = np.zeros((T, M), np.float16)
        mx[:, :P] = ((t >= lo[a][None, :]) & (t < hi[a][None, :]))
        mx[:, P:] = ((t - sa >= lo[b][None, :]) & (t - sa < hi[b][None, :])
                     & (t >= sa))
        ep = np.zeros((Tc, H), np.float16)
        ep[:ca] = emb16[ids[a, :ca]]
        ep[ca:ca + cb] = emb16[ids[b, :cb]]
        cm = np.zeros((Tc, SPC), np.float16)
        cm[:ca, 0] = 1.0
        cm[ca:ca + cb, 1] = 1.0
        aug = np.zeros((4, M), np.float32)
        aug[0, :P] = ind[a] / ca
        aug[1, P:] = ind[b] / cb
        aug[2, :P] = cnt[a]
        aug[2, P:] = cnt[b]
        aug[3, :P] = ind[a]
        aug[3, P:] = ind[b]
        sinv = np.concatenate([1.0 / Sj[a], 1.0 / Sj[b]]).reshape(M, 1)
        in_maps.append({
            "xh": xp, "mxr": _pm(mx, KT), "eh": ep, "cmr": _pm(cm, KC),
            "wcr": wcr, "wdr": wdr, "wur": wur,
            "bdr": bd_r, "bcr": bc_r, "bur": bu_r,
            "aug": aug, "sinv": sinv.astype(np.float32),
        })

    res = run_bass_kernel_spmd(nc, in_maps, core_ids=list(range(NC)))
    _cache["last_result"] = res

    out = np.empty((B, P, H), np.float32)
    for c, (a, b) in enumerate(pairs):
        o = res.results[c]["out"]
        out[a] = o[:P]
        out[b] = o[P:]
    return out


# revision 12
# speedup vs baseline: 1.2898x; 1.2898x over previous
"""Trainium2 Bass kernel for nn_ContextPromptGenerator.

Math restructure: pooled bins are masked segment sums over tokens, so the
0/1-mask matmul runs FIRST on [T, 4096] packed rows, then the 4096->1024
down-projection runs on [64, 4096] per core.  All operands are plain fp16
(fp32 PSUM): the error gate is 2e-2 and fp16 lands ~4e-4.

Phase X keeps x stationary and streams the 64-bin mask (the orientation-B
variant with mask stationary was tried and regressed: its [64, 4096] psum
tile owns all 8 banks, forcing phases A/B off the interleaved stream and
serializing ~55us of work that is otherwise hidden under X).

Schedule: A (ctx sums) -> X (bin sums) with B (ctx @ Wc) interleaved so
Wc streams during X and B's matmuls hide in X's tensor stream -> D (@Wd
resident + aug, silu) -> E (transpose) -> U (@Wu + bu, halves
double-buffered).

DMA: weights/masks host-reordered partition-major so partition lines are
contiguous multi-KB runs (8KB packets sustain ~430GB/s vs ~170GB/s at 2KB).

Sharding: data-parallel, 2 samples per core, paired to minimize
roundup128(max seq pair) + roundup128(max ctx pair).
"""

import numpy as np
from contextlib import ExitStack

import concourse.bass as bass
import concourse.mybir as mybir
import concourse.tile as tile
from concourse import bacc
from concourse.masks import make_identity
from concourse.bass_utils import run_bass_kernel_spmd

F32 = mybir.dt.float32
F16 = mybir.dt.float16

B, S, C, H, D, V, P = 16, 2048, 512, 4096, 1024, 32000, 32
NC = 8          # cores
SPC = 2         # samples per core
M = SPC * P     # 64 output rows per core
HT = H // 128   # 32 h-tiles
DT = D // 128   # 8 d-tiles

_cache = {}


def _build(T, Tc):
    """Build the per-core SPMD Bass program.

    T  = packed hidden rows per core (multiple of 128)
    Tc = packed context-embedding rows per core (multiple of 128)
    """
    nc = bacc.Bacc(None, target_bir_lowering=False)

    KT = T // 128    # x k-tiles
    KC = Tc // 128   # emb k-tiles

    xh_d = nc.dram_tensor("xh", [T, H], F16, kind="ExternalInput")
    mx_d = nc.dram_tensor("mxr", [128, KT * M], F16, kind="ExternalInput")
    eh_d = nc.dram_tensor("eh", [Tc, H], F16, kind="ExternalInput")
    cm_d = nc.dram_tensor("cmr", [128, KC * SPC], F16, kind="ExternalInput")
    wch_d = nc.dram_tensor("wcr", [128, HT * D], F16, kind="ExternalInput")
    wdh_d = nc.dram_tensor("wdr", [128, HT * D], F16, kind="ExternalInput")
    wuh_d = nc.dram_tensor("wur", [128, 2 * DT * H // 2], F16,
                           kind="ExternalInput")
    bd_d = nc.dram_tensor("bdr", [1, D], F32, kind="ExternalInput")
    bc_d = nc.dram_tensor("bcr", [1, D], F32, kind="ExternalInput")
    bu_d = nc.dram_tensor("bur", [1, H], F16, kind="ExternalInput")
    aug_d = nc.dram_tensor("aug", [4, M], F32, kind="ExternalInput")
    sinv_d = nc.dram_tensor("sinv", [M, 1], F32, kind="ExternalInput")
    out_d = nc.dram_tensor("out", [M, H], F32, kind="ExternalOutput")

    WCC = 8 * D       # Wc/Wd chunk cols (8 k-tiles per chunk)
    NCHUNK = HT // 8  # 4 chunks

    with tile.TileContext(nc) as tc, ExitStack() as ctx:
        const = ctx.enter_context(tc.tile_pool(name="const", bufs=1))
        big = ctx.enter_context(tc.tile_pool(name="big", bufs=2))
        wcpool = ctx.enter_context(tc.tile_pool(name="wcpool", bufs=1))
        wupool = ctx.enter_context(tc.tile_pool(name="wupool", bufs=2))
        opool = ctx.enter_context(tc.tile_pool(name="opool", bufs=1))
        keep = ctx.enter_context(tc.tile_pool(name="keep", bufs=1))

        ident16 = const.tile([128, 128], F16)
        idtmp = const.tile([128, 128], F32)
        make_identity(nc, idtmp)
        nc.vector.tensor_copy(ident16, idtmp)
        ones1 = const.tile([1, M], F16)
        nc.vector.memset(ones1, 1.0)
        aug_sb = keep.tile([4, M], F32)
        nc.sync.dma_start(out=aug_sb, in_=aug_d[:, :])
        sinv_sb = keep.tile([M, 1], F32)
        nc.sync.dma_start(out=sinv_sb, in_=sinv_d[:, :])
        # augmented-rhs rows: 0=ctxWcSum[a], 1=ctxWcSum[b], 2=bd, 3=bc
        augr_sb = keep.tile([4, D], F32)
        nc.sync.dma_start(out=augr_sb[2:3, :], in_=bd_d[:, :])
        nc.sync.dma_start(out=augr_sb[3:4, :], in_=bc_d[:, :])
        mxr_sb = keep.tile([128, KT * M], F16)
        nc.sync.dma_start(out=mxr_sb, in_=mx_d[:, :])
        cmr_sb = keep.tile([128, KC * SPC], F16)
        nc.sync.dma_start(out=cmr_sb, in_=cm_d[:, :])
        wd_sb = keep.tile([128, HT * D], F16)   # resident Wd, 64KB/part

        # ---- phase A: ctx_sumT[h, s] = sum_r emb[r, h] * cm01[r, s] ----
        cs_hi = keep.tile([128, HT * SPC], F16)
        with tc.tile_pool(name="psA", bufs=1, space="PSUM") as psA:
            ps_ctx = psA.tile([128, HT * SPC], F32)  # 1 bank
            for k in range(KC):
                eht = big.tile([128, H], F16, tag="hih")
                nc.sync.dma_start(out=eht, in_=eh_d[128 * k:128 * (k + 1), :])
                for hc in range(HT):
                    nc.tensor.matmul(
                        ps_ctx[:, SPC * hc:SPC * (hc + 1)],
                        eht[:, 128 * hc:128 * (hc + 1)],
                        cmr_sb[:, SPC * k:SPC * (k + 1)],
                        start=(k == 0 and hc == 0),
                        stop=(k == KC - 1),
                    )
            nc.vector.tensor_copy(cs_hi, ps_ctx)

        # ---- phase X: xsumT[h, j] = sum_t x[t, h] * mx01[t, j] ----
        # x tiles stationary, 0/1 mask moving; out 32 slices [128,64].
        # Phase B (ctxWcSum rows [2,1024] = cs.T @ Wc) is interleaved: Wc
        # chunk c's DMA at X-iter 3c, its 16 matmuls at X-iter 3c+2, so the
        # Wc stream overlaps x DMA and B's matmuls hide inside X's stream.
        wc_tiles = {}
        state = {"dma": 0, "mm": 0}

        def emit_wc_dma():
            c = state["dma"]
            if c < NCHUNK:
                wct = wcpool.tile([128, WCC], F16, tag="wc")
                nc.sync.dma_start(out=wct, in_=wch_d[:, WCC * c:WCC * (c + 1)])
                wc_tiles[c] = wct
                state["dma"] = c + 1

        def emit_b_chunk():
            c = state["mm"]
            if c < NCHUNK and c < state["dma"]:
                wct = wc_tiles.pop(c)
                for kk in range(8):
                    k = 8 * c + kk
                    for nb in range(2):
                        nc.tensor.matmul(
                            ps_cd[:, 512 * nb:512 * (nb + 1)],
                            cs_hi[:, SPC * k:SPC * (k + 1)],
                            wct[:, D * kk + 512 * nb:D * kk + 512 * (nb + 1)],
                            start=(k == 0),
                            stop=(k == HT - 1),
                        )
                state["mm"] = c + 1

        xs_hi = keep.tile([128, HT * M], F16)
        with tc.tile_pool(name="psX", bufs=1, space="PSUM") as psX, \
                tc.tile_pool(name="psB", bufs=1, space="PSUM") as psB:
            ps_xs = psX.tile([128, HT * M], F32)  # 4 banks, 8 slices per bank
            ps_cd = psB.tile([SPC, D], F32)       # 2 banks
            for k in range(KT):
                xht = big.tile([128, H], F16, tag="hih")
                nc.sync.dma_start(out=xht, in_=xh_d[128 * k:128 * (k + 1), :])
                if k % 3 == 0:
                    emit_wc_dma()
                for hc in range(HT):
                    nc.tensor.matmul(
                        ps_xs[:, M * hc:M * (hc + 1)],
                        xht[:, 128 * hc:128 * (hc + 1)],
                        mxr_sb[:, M * k:M * (k + 1)],
                        start=(k == 0 and hc % 8 == 0),
                        stop=(k == KT - 1),
                    )
                if k % 3 == 2:
                    emit_b_chunk()
            # resident Wd lands while X's tensor stream drains
            for c in range(NCHUNK):
                nc.sync.dma_start(
                    out=wd_sb[:, WCC * c:WCC * (c + 1)],
                    in_=wdh_d[:, WCC * c:WCC * (c + 1)])
            while state["mm"] < NCHUNK:
                emit_wc_dma()
                emit_b_chunk()
            for q in range(4):
                nc.vector.tensor_copy(
                    xs_hi[:, 512 * q:512 * (q + 1)],
                    ps_xs[:, 512 * q:512 * (q + 1)])
            nc.vector.tensor_copy(augr_sb[0:2, :], ps_cd)

        # ---- phase D: pooled[j, d] = xsum.T @ Wd + aug ----
        silu_sb = keep.tile([M, D], F16)
        with tc.tile_pool(name="psD", bufs=1, space="PSUM") as psD:
            ps_pool = psD.tile([M, D], F32)  # 2 banks
            for k in range(HT):
                for nb in range(2):
                    nc.tensor.matmul(
                        ps_pool[:, 512 * nb:512 * (nb + 1)],
                        xs_hi[:, M * k:M * (k + 1)],
                        wd_sb[:, D * k + 512 * nb:D * k + 512 * (nb + 1)],
                        start=(k == 0),
                        stop=False,
                    )
            for nb in range(2):
                nc.tensor.matmul(
                    ps_pool[:, 512 * nb:512 * (nb + 1)],
                    aug_sb,
                    augr_sb[:, 512 * nb:512 * (nb + 1)],
                    start=False, stop=True,
                )
            # scale by 1/S and silu in one ACT op per bank
            for nb in range(2):
                nc.scalar.activation(
                    silu_sb[:, 512 * nb:512 * (nb + 1)],
                    ps_pool[:, 512 * nb:512 * (nb + 1)],
                    mybir.ActivationFunctionType.Silu,
                    scale=sinv_sb,
                )

        # ---- phase E: siluT slices [128, 64] per d-tile ----
        sT_hi = keep.tile([128, DT * M], F16)
        with tc.tile_pool(name="psE", bufs=2, space="PSUM") as psE:
            for dc in range(DT):
                pst = psE.tile([128, M], F16, tag="silutr")
                nc.tensor.transpose(
                    pst, silu_sb[:, 128 * dc:128 * (dc + 1)],
                    ident16[0:M, 0:M])
                nc.vector.tensor_copy(sT_hi[:, M * dc:M * (dc + 1)], pst)

        # ---- phase U: out[j, h] = siluT.T @ Wu + bu ----
        HH = H // 2  # two halves to keep psum at 4 banks
        with tc.tile_pool(name="psU", bufs=2, space="PSUM") as psU:
            for half in range(2):
                wut = wupool.tile([128, DT * HH], F16, tag="wuh")  # 32KB/part
                for c in range(2):
                    nc.sync.dma_start(
                        out=wut[:, DT * HH // 2 * c:DT * HH // 2 * (c + 1)],
                        in_=wuh_d[:, DT * HH * half + DT * HH // 2 * c:
                                  DT * HH * half + DT * HH // 2 * (c + 1)])
                but = opool.tile([1, HH], F16, tag="bu")
                nc.sync.dma_start(
                    out=but, in_=bu_d[:, HH * half:HH * (half + 1)])
                ps_out = psU.tile([M, HH], F32, tag="outps")  # 4 banks
                for dc in range(DT):
                    for nb in range(HH // 512):
                        nc.tensor.matmul(
                            ps_out[:, 512 * nb:512 * (nb + 1)],
                            sT_hi[:, M * dc:M * (dc + 1)],
                            wut[:, HH * dc + 512 * nb:HH * dc + 512 * (nb + 1)],
                            start=(dc == 0),
                            stop=False,
                        )
                for nb in range(HH // 512):
                    nc.tensor.matmul(
                        ps_out[:, 512 * nb:512 * (nb + 1)],
                        ones1,
                        but[:, 512 * nb:512 * (nb + 1)],
                        start=False, stop=True,
                    )
                ot = opool.tile([M, HH], F32, tag="ot")
                for nb in range(HH // 512):
                    nc.vector.tensor_copy(
                        ot[:, 512 * nb:512 * (nb + 1)],
                        ps_out[:, 512 * nb:512 * (nb + 1)])
                nc.sync.dma_start(
                    out=out_d[:, HH * half:HH * (half + 1)], in_=ot)

    nc.finalize()
    return nc


def _roundup(v, m):
    return max(m, ((int(v) + m - 1) // m) * m)


def _pm(a, kt):
    """Reorder [kt*128, cols] row-major -> partition-major [128, kt*cols]."""
    n, cols = a.shape
    assert n == kt * 128
    return np.ascontiguousarray(
        a.reshape(kt, 128, cols).transpose(1, 0, 2).reshape(128, kt * cols))


def _pair_samples(seq, clen):
    """Pair the 16 samples 2-per-core minimizing packed-row DMA:
    roundup128(max pair seq) + roundup128(max pair clen), tie-broken by joint
    row count.  Greedy sort-and-reflect on seq+clen, then 2-opt passes."""
    w = seq + clen
    order = np.argsort(-w, kind="stable")
    pairs = [[int(order[i]), int(order[2 * NC - 1 - i])] for i in range(NC)]

    def cost(ps):
        tx = max(seq[a] + seq[b] for a, b in ps)
        te = max(clen[a] + clen[b] for a, b in ps)
        return (_roundup(tx, 128) + _roundup(te, 128),
                max(seq[a] + clen[a] + seq[b] + clen[b] for a, b in ps))

    best = cost(pairs)
    improved = True
    while improved:
        improved = False
        for i in range(NC):
            for j in range(i + 1, NC):
                for swap in ((1, 1), (1, 0), (0, 1)):
                    cand = [list(p) for p in pairs]
                    cand[i][swap[0]], cand[j][swap[1]] = \
                        cand[j][swap[1]], cand[i][swap[0]]
                    c = cost(cand)
                    if c < best:
                        best, pairs, improved = c, cand, True
    return [(a, b) for a, b in pairs]


def kernel(**inputs):
    ids = np.asarray(inputs["context_ids"]).astype(np.int64)
    x = np.asarray(inputs["hidden_states"], dtype=np.float32)
    seq = np.asarray(inputs["seq_lengths"]).astype(np.int64)
    clen = np.asarray(inputs["context_lengths"]).astype(np.int64)
    emb = np.asarray(inputs["embed_table"], dtype=np.float32)
    Wc = np.ascontiguousarray(inputs["Wc"], dtype=np.float32)
    bc = np.asarray(inputs["bc"], dtype=np.float32)
    Wd = np.ascontiguousarray(inputs["Wd"], dtype=np.float32)
    bd = np.asarray(inputs["bd"], dtype=np.float32)
    Wu = np.ascontiguousarray(inputs["Wu"], dtype=np.float32)
    bu = np.asarray(inputs["bu"], dtype=np.float32)

    assert x.shape == (B, S, H) and ids.shape == (B, C)

    # per-sample bin geometry
    L = seq + 1
    jj = np.arange(P, dtype=np.int64)
    start = (jj[None, :] * L[:, None]) // P            # [B,P]
    end = ((jj[None, :] + 1) * L[:, None] + P - 1) // P
    Sj = (end - start).astype(np.float32)
    lo = np.maximum(start - 1, 0)
    hi = end - 1
    cnt = (hi - lo).astype(np.float32)
    ind = (start == 0).astype(np.float32)

    pairs = _pair_samples(seq, clen)
    T = _roundup(max(seq[a] + seq[b] for a, b in pairs), 128)
    Tc = _roundup(max(clen[a] + clen[b] for a, b in pairs), 128)
    KT, KC = T // 128, Tc // 128

    key = (T, Tc)
    if key not in _cache:
        _cache[key] = _build(T, Tc)
    nc = _cache[key]

    # partition-major weight layouts (8KB+ DMA packets)
    wcr = _pm(Wc.astype(np.float16), HT)
    wdr = _pm(Wd.astype(np.float16), HT)
    # Wu: [1024, 4096] -> [128, (half, dc, 2048)]
    wur = np.ascontiguousarray(
        Wu.astype(np.float16).reshape(DT, 128, 2, H // 2)
        .transpose(1, 2, 0, 3).reshape(128, 2 * DT * (H // 2)))
    bd_r = bd.reshape(1, D)
    bc_r = bc.reshape(1, D)
    bu_r = bu.astype(np.float16).reshape(1, H)
    emb16 = emb.astype(np.float16)

    in_maps = []
    for a, b in pairs:
        sa, sb = int(seq[a]), int(seq[b])
        ca, cb = max(1, int(clen[a])), max(1, int(clen[b]))
        xp = np.zeros((T, H), np.float16)
        xp[:sa] = x[a, :sa]
        xp[sa:sa + sb] = x[b, :sb]
        t = np.arange(T, dtype=np.int64)[:, None]
        mx = np.zeros((T, M), np.float16)
        mx[:, :P] = ((t >= lo[a][None, :]) & (t < hi[a][None, :]))
        mx[:, P:] = ((t - sa >= lo[b][None, :]) & (t - sa < hi[b][None, :])
                     & (t >= sa))
        ep = np.zeros((Tc, H), np.float16)
        ep[:ca] = emb16[ids[a, :ca]]
        ep[ca:ca + cb] = emb16[ids[b, :cb]]
        cm = np.zeros((Tc, SPC), np.float16)
        cm[:ca, 0] = 1.0
        cm[ca:ca + cb, 1] = 1.0
        aug = np.zeros((4, M), np.float32)
        aug[0, :P] = ind[a] / ca
        aug[1, P:] = ind[b] / cb
        aug[2, :P] = cnt[a]
        aug[2, P:] = cnt[b]
        aug[3, :P] = ind[a]
        aug[3, P:] = ind[b]
        sinv = np.concatenate([1.0 / Sj[a], 1.0 / Sj[b]]).reshape(M, 1)
        in_maps.append({
            "xh": xp, "mxr": _pm(mx, KT), "eh": ep, "cmr": _pm(cm, KC),
            "wcr": wcr, "wdr": wdr, "wur": wur,
            "bdr": bd_r, "bcr": bc_r, "bur": bu_r,
            "aug": aug, "sinv": sinv.astype(np.float32),
        })

    res = run_bass_kernel_spmd(nc, in_maps, core_ids=list(range(NC)))
    _cache["last_result"] = res

    out = np.empty((B, P, H), np.float32)
    for c, (a, b) in enumerate(pairs):
        o = res.results[c]["out"]
        out[a] = o[:P]
        out[b] = o[P:]
    return out
